# revision 19
# baseline (speedup 1.0000x reference)
"""Trainium2 Bass kernel for nn_BernoulliDecompAttModel (decomposable attention NLI model).

Contract: kernel(**inputs) takes the FULL unsharded inputs (as produced by
setup_inputs()) and returns the FULL [64, 3] float32 output. Internally the
batch (64) is sharded 8-ways across 8 NeuronCores (pure data parallel, all
weights replicated); each core runs an identical Bass/Tile program on its 8
batch items.

Final version (963us -> 409us on HW): fp8-e4m3 operands + DoubleRow perf
mode (two contraction chunks per pass, ~2x PE throughput, verified by
microbenchmark) on the numerically-tolerant matmuls: self/inter MLPs,
compare-L1 (+Y), ctx, attended, q.q / pq.hk scores. Weight-stationary MLP
matmuls use DoubleRowSwInterleave with host-interleaved weights. fp16
operands elsewhere (cmp2, aggregate head — fp8 there pushes the error past
the harness gate; measured on CPU with the real reference data). All PSUM
accumulation and softmax staging stays fp32; softmax skips max-subtraction
(scores provably bounded, exp cannot overflow f32). Embedding gather is
fp8 direct (transport verified bit-exact). Measured rel err 7.6e-3 vs the
2e-2 gate.

Layout conventions inside the device program (per core):
  - Activations are kept "feature-major": [128 partitions = feature chunk,
    K/128 chunks, token axis on free dim]. Matmul contracts over partitions,
    so feature-major activations feed matmuls directly with weights stored
    natural [Kin (partitions x chunks), Nout]. DoubleRow consumes two
    adjacent K-chunks per instruction: lhsT [128, 2, M], rhs [128, 2, N].
  - prem and hypo (256 tokens each) are concatenated on the token axis for
    every shared-weight MLP -> N=512 matmuls.
  - Attention: scores accumulate in PSUM; key masks are injected by a K=1
    matmul ones[1,128] (x) madd[1,256] accumulated into the same PSUM bank;
    the relative-distance bias (a Toeplitz matrix) is built once from a
    511-value strip via M[i',j] = strip[i'+j] (dense DMA) followed by a
    partition-flip matmul with the anti-identity J; strip[255] = -30000
    bakes the score-diagonal -inf of the reference (fp16-safe).
  - The z^T branch keeps the f32r-transpose trick (4-byte PSUM elements) so
    the prem-key mask matmul (f32 out) can share its PSUM bank.
"""

import numpy as np
import os

B, L, V, E, D, OUT = 64, 256, 50000, 512, 512, 3
NCORES = 8
BL = B // NCORES            # batch items per core
MAX_DIST = 11
MASK_VAL = -30000.0         # padded-key additive mask (exp() underflows to 0)
DIAG_VAL = -30000.0         # self-attention diagonal (fp16-safe; exp() -> 0)

_PROG_CACHE = {}


def _build_program(debug_taps=()):
    import concourse.bass as bass
    import concourse.bacc as bacc
    import concourse.mybir as mybir
    from concourse.tile import TileContext
    from concourse.masks import make_identity

    dt = mybir.dt
    f32, f32r, i32 = dt.float32, dt.float32r, dt.int32
    hf, f8 = dt.float16, dt.float8e4
    DR = mybir.MatmulPerfMode.DoubleRow
    DRI = mybir.MatmulPerfMode.DoubleRowSwInterleave
    AF = mybir.ActivationFunctionType
    ALU = mybir.AluOpType
    AX = mybir.AxisListType

    nc = bacc.Bacc("TRN2", target_bir_lowering=False, debug=True)

    # ---------------- DRAM I/O ----------------
    tok = nc.dram_tensor("tok", [2, BL, L], i32, kind="ExternalInput")
    emb = nc.dram_tensor("emb", [V, E], f8, kind="ExternalInput")
    dW = nc.dram_tensor("dW", [2 * MAX_DIST + 1], f32, kind="ExternalInput")
    w_s1 = nc.dram_tensor("w_s1", [128, E // 256, 4, 256], f8, kind="ExternalInput")
    w_s2 = nc.dram_tensor("w_s2", [128, D // 256, 4, 256], f8, kind="ExternalInput")
    w_a1 = nc.dram_tensor("w_a1", [128, 2 * E // 256, 4, 256], f8, kind="ExternalInput")
    w_a2 = nc.dram_tensor("w_a2", [128, D // 256, 4, 256], f8, kind="ExternalInput")
    w_c1t = nc.dram_tensor("w_c1t", [128, 2 * E // 256, 4, 256], f8, kind="ExternalInput")
    w_c1b = nc.dram_tensor("w_c1b", [2 * E, D], f8, kind="ExternalInput")
    w_c2 = nc.dram_tensor("w_c2", [D, D], hf, kind="ExternalInput")
    w_g1 = nc.dram_tensor("w_g1", [2 * D, D], hf, kind="ExternalInput")
    w_g2 = nc.dram_tensor("w_g2", [D, D], hf, kind="ExternalInput")
    w_o = nc.dram_tensor("w_o", [D, OUT], hf, kind="ExternalInput")
    bias_names = ["b_s1", "b_s2", "b_a1", "b_a2", "b_c1", "b_c2", "b_g1", "b_g2"]
    bdram = {n: nc.dram_tensor(n, [D], f32, kind="ExternalInput") for n in bias_names}

    out_d = nc.dram_tensor("out", [BL, OUT], f32, kind="ExternalOutput")

    strip_d = nc.dram_tensor("strip_d", [2 * L - 1], hf)

    dbg = {}
    for name, shape in debug_taps:
        dbg[name] = nc.dram_tensor(name, shape, f32, kind="ExternalOutput")

    with TileContext(nc) as tc:
        const = tc.alloc_tile_pool(name="const", bufs=1)
        work = tc.alloc_tile_pool(name="work", bufs=2)
        ps = tc.alloc_tile_pool(name="ps", bufs=1, space="PSUM")

        def mm512_ps(name):
            return ps.tile([128, 512], f32, space="PSUM", tag="mm512", bufs=3, name=name)

        def attn_ps(name):
            return ps.tile([128, 256], f32, space="PSUM", tag="attn", bufs=3, name=name)

        def trT_ps(name, dtype):
            return ps.tile([128, 128], dtype, space="PSUM", tag="trT", bufs=2, name=name)

        # ---------------- constants / weights ----------------
        def load_w(dram, K, N, dtype, name):
            t = const.tile([128, K // 128, N], dtype, name=name)
            nc.sync.dma_start(t[:], bass.AP(dram, 0, [[N, 128], [128 * N, K // 128], [1, N]]))
            return t

        def load_dri(dram, K, name):
            t = const.tile([128, K // 256, 4, 256], f8, name=name)
            nc.sync.dma_start(t[:], dram.ap())
            return t

        ws1 = load_dri(w_s1, E, "ws1")
        ws2 = load_dri(w_s2, D, "ws2")
        wa1 = load_dri(w_a1, 2 * E, "wa1")
        wa2 = load_dri(w_a2, D, "wa2")
        wc1t = load_dri(w_c1t, 2 * E, "wc1t")
        wc1b = load_w(w_c1b, 2 * E, D, f8, "wc1b")
        wc2 = load_w(w_c2, D, D, hf, "wc2")
        wg1 = load_w(w_g1, 2 * D, D, hf, "wg1")
        wg2 = load_w(w_g2, D, D, hf, "wg2")
        wo = const.tile([128, 4, 4], hf, name="wo")
        nc.vector.memset(wo[:], 0.0)
        nc.sync.dma_start(wo[:, :, 0:OUT], bass.AP(w_o, 0, [[OUT, 128], [128 * OUT, 4], [1, OUT]]))

        bsb = {}
        for n in bias_names:
            t = const.tile([128, 4], f32, name=f"sb_{n}")
            nc.sync.dma_start(t[:], bass.AP(bdram[n], 0, [[1, 128], [128, 4]]))
            bsb[n] = t
        bc2row32 = const.tile([1, D], f32, name="bc2row32")
        nc.sync.dma_start(bc2row32[:], bass.AP(bdram["b_c2"], 0, [[0, 1], [1, D]]))
        bc2row = const.tile([1, D], hf, name="bc2row")
        nc.vector.tensor_copy(bc2row[:], bc2row32[:])

        ones32 = const.tile([1, 128], f32, name="ones32")
        nc.vector.memset(ones32[:], 1.0)
        ones = const.tile([1, 128], hf, name="ones")
        nc.vector.tensor_copy(ones[:], ones32[:])

        ident32 = const.tile([128, 128], f32, name="ident32")
        make_identity(nc, ident32[:])
        identb = const.tile([128, 128], hf, name="identb")
        nc.vector.tensor_copy(identb[:], ident32[:])
        ident8 = const.tile([128, 128], f8, name="ident8")
        nc.vector.tensor_copy(ident8[:], ident32[:])
        identr = const.tile([128, 128], f32r, name="identr")
        nc.vector.tensor_copy(identr[:], ident32[:])

        # anti-identity J[p,f] = 1 iff p + f == 127
        J32 = const.tile([128, 128], f32, name="J32")
        nc.gpsimd.memset(J32[:], 0.0)
        nc.gpsimd.affine_select(
            out=J32[:], in_=J32[:], compare_op=ALU.not_equal, fill=1.0,
            base=-127, pattern=[[1, 128]], channel_multiplier=1,
        )
        J = const.tile([128, 128], hf, name="J")
        nc.vector.tensor_copy(J[:], J32[:])

        # ---------------- relative-distance bias matrix ----------------
        # strip[d] = dW[clip(d-255, -11, 11) + 11], strip[255] = DIAG_VAL
        dwsb = const.tile([1, 2 * MAX_DIST + 1], f32, name="dwsb")
        nc.sync.dma_start(dwsb[:], bass.AP(dW, 0, [[0, 1], [1, 2 * MAX_DIST + 1]]))
        strip = const.tile([1, 2 * L - 1], f32, name="strip")
        lo = L - 1 - MAX_DIST          # 244
        hi = L - 1 + MAX_DIST          # 266
        nc.vector.tensor_copy(strip[:, 0:lo], dwsb[:, 0:1].to_broadcast([1, lo]))
        nc.vector.tensor_copy(strip[:, lo:hi + 1], dwsb[:, :])
        nc.vector.tensor_copy(strip[:, hi + 1:2 * L - 1],
                              dwsb[:, 2 * MAX_DIST:2 * MAX_DIST + 1].to_broadcast([1, 2 * L - 2 - hi]))
        nc.vector.memset(strip[:, L - 1:L], DIAG_VAL)
        stripb = const.tile([1, 2 * L - 1], hf, name="stripb")
        nc.vector.tensor_copy(stripb[:], strip[:])
        nc.sync.dma_start(bass.AP(strip_d, 0, [[0, 1], [1, 2 * L - 1]]), stripb[:])
        # M[i',j] = strip[i'+j]; bias[i,j] = M[255-i, j] via J-flip matmul
        Msb = const.tile([128, 2, 256], hf, name="Msb")
        for mc in range(2):
            nc.sync.dma_start(Msb[:, mc, :], bass.AP(strip_d, 128 * mc, [[1, 128], [1, 256]]))
        bias_sb = const.tile([128, 2, 256], f32, name="bias_sb")
        for ic in range(2):
            pb = attn_ps(f"biasflip{ic}")
            nc.tensor.matmul(pb[:], lhsT=J[:], rhs=Msb[:, 1 - ic, :], start=True, stop=True)
            nc.vector.tensor_copy(bias_sb[:, ic, :], pb[:])

        # ---------------- per-item pipeline ----------------
        def softmax_rows(src_ap, dst_ap, tag_suffix):
            """row softmax: src_ap [128,256] (SBUF or PSUM, f32 view) -> dst_ap

            No max-subtraction: raw scores here are bounded (|q.q| <= ~6,
            masked entries -30000 -> exp underflows to 0), so exp() cannot
            overflow f32 and the result is mathematically identical."""
            esum = work.tile([128, 1], f32, tag="esum", bufs=4, name=f"esum{tag_suffix}")
            nc.scalar.activation(dst_ap, src_ap, AF.Exp, bias=0.0, scale=1.0,
                                 accum_out=esum[:])
            rec = work.tile([128, 1], f32, tag="rec", bufs=4, name=f"rec{tag_suffix}")
            nc.vector.reciprocal(rec[:], esum[:])
            nc.vector.tensor_scalar(dst_ap, dst_ap, rec[:, 0:1], None, op0=ALU.mult)

        srows = const.tile([2 * BL, 512], f32r, name="srows")

        nitems = int(os.environ.get('KITEMS', BL))
        STAGE = int(os.environ.get('KSTAGE', 99))

        # ---- prologue: all items' indices, masks, embedding gathers ----
        # (gathers drain the DMA queues early so per-item transposes never
        # wait on them)
        its, maskfs, madds, xembss = [], [], [], []
        for b in range(nitems):
            it = work.tile([128, 2, 2], i32, tag="it", bufs=BL, name=f"it{b}")
            for s in range(2):
                nc.sync.dma_start(it[:, s, :], bass.AP(tok, b * L + s * BL * L, [[1, 128], [128, 2]]))
            idxrow = work.tile([1, 2, L], i32, tag="idxrow", bufs=2, name=f"idxrow{b}")
            for s in range(2):
                nc.sync.dma_start(idxrow[:, s, :], bass.AP(tok, b * L + s * BL * L, [[0, 1], [1, L]]))
            maskf = work.tile([128, 2, 2, 1], f8, tag="maskf", bufs=BL, name=f"maskf{b}")
            nc.vector.tensor_scalar(maskf[:], it[:], 0, None, op0=ALU.not_equal)
            madd = work.tile([1, 2, L], hf, tag="madd", bufs=BL, name=f"madd{b}")
            nc.vector.tensor_scalar(madd[:], idxrow[:], 0, MASK_VAL,
                                    op0=ALU.is_equal, op1=ALU.mult)
            xembs = [work.tile([128, 2, E], f8, tag=f"xembs{s}", bufs=BL, name=f"xembs{b}_{s}")
                     for s in range(2)]
            for s in range(2):
                for tcn in range(2):
                    nc.gpsimd.indirect_dma_start(
                        out=xembs[s][:, tcn, :], out_offset=None, in_=emb.ap(),
                        in_offset=bass.IndirectOffsetOnAxis(ap=it[:, s, tcn:tcn + 1], axis=0))
            its.append(it)
            maskfs.append(maskf)
            madds.append(madd)
            xembss.append(xembs)

        for b in range(nitems):
            it, maskf, madd, xembs = its[b], maskfs[b], madds[b], xembss[b]

            if STAGE < 2:
                continue
            # ---- x transposes -> cmpin kc 0..3 (feature-major cat, both seqs) ----
            # (fp8 is_transpose is rejected by the walrus verifier; a normal
            # matmul against the identity computes the exact transpose at the
            # same 128-column cost, accumulating in f32)
            cmpin = work.tile([128, 8, 512], f8, tag="cmpin", bufs=3, name=f"cmpin{b}")
            for s in range(2):
                for tcn in range(2):
                    for dc in range(4):
                        ptr = trT_ps(f"xT{b}_{s}{tcn}{dc}", f32)
                        nc.tensor.matmul(ptr[:], lhsT=xembs[s][:, tcn, dc * 128:(dc + 1) * 128],
                                         rhs=ident8[:], start=True, stop=True)
                        dst = cmpin[:, dc, s * 256 + tcn * 128:s * 256 + (tcn + 1) * 128]
                        if dc % 2 == 0:
                            nc.vector.tensor_copy(dst, ptr[:])
                        else:
                            nc.scalar.copy(dst, ptr[:])

            if STAGE < 3:
                continue
            # ---- MLP layer, DoubleRow fp8: two K-chunks per matmul ----
            def mlp_fm(src, nkc, w, bias_t, dst, name):
                """feature-major MLP layer: dst[:,nf,:] = relu(w.T @ src + bias).
                w is in the DRI interleaved layout [128, pairs, 4, 256]."""
                npair = nkc // 2
                for nf in range(4):
                    pm = mm512_ps(f"{name}_nf{nf}")
                    for i in range(npair):
                        nc.tensor.matmul(pm[:], lhsT=w[:, i, nf, :],
                                         rhs=src[:, 2 * i:2 * i + 2, :],
                                         start=(i == 0), stop=(i == npair - 1), perf_mode=DRI)
                    nc.scalar.activation(dst[:, nf, :], pm[:], AF.Relu, bias=bias_t[:, nf:nf + 1])

            hmid = work.tile([128, 4, 512], f8, tag="mid", bufs=3, name=f"h1_{b}")
            mlp_fm(cmpin, 4, ws1, bsb["b_s1"], hmid, f"sm1_{b}")
            qb = work.tile([128, 4, 512], f8, tag="qpq", bufs=3, name=f"q_{b}")
            mlp_fm(hmid, 4, ws2, bsb["b_s2"], qb, f"sm2_{b}")

            if STAGE < 4:
                continue
            # ---- self attention per sequence (DR scores on fp8 q) ----
            att = work.tile([128, 4, 256], hf, tag="att", bufs=3, name=f"att{b}")
            for s in range(2):
                for ic in range(2):
                    pS = attn_ps(f"S{b}_{s}{ic}")
                    nc.tensor.matmul(pS[:], lhsT=ones[:], rhs=madd[0:1, s, :],
                                     start=True, stop=False, skip_group_check=True)
                    for kp in range(2):
                        nc.tensor.matmul(pS[:], lhsT=qb[:, 2 * kp:2 * kp + 2, s * 256 + ic * 128:s * 256 + (ic + 1) * 128],
                                         rhs=qb[:, 2 * kp:2 * kp + 2, s * 256:(s + 1) * 256],
                                         start=False, stop=(kp == 1), perf_mode=DR,
                                         skip_group_check=True)
                    sm = work.tile([128, 256], f32, tag="sm", bufs=3, name=f"sm{b}_{s}{ic}")
                    nc.vector.tensor_tensor(sm[:], pS[:], bias_sb[:, ic, :], op=ALU.add)
                    softmax_rows(sm[:], att[:, s * 2 + ic, :], f"_att{b}_{s}{ic}")

            if STAGE < 5:
                continue
            attT = work.tile([128, 4, 256], f8, tag="attT", bufs=3, name=f"attT{b}")
            for s in range(2):
                for jc in range(2):
                    for ic in range(2):
                        ptr = trT_ps(f"attT{b}_{s}{jc}{ic}", hf)
                        nc.tensor.matmul(ptr[:], lhsT=att[:, s * 2 + ic, jc * 128:(jc + 1) * 128],
                                         rhs=identb[:], is_transpose=True, start=True, stop=True)
                        nc.vector.tensor_copy(attT[:, s * 2 + jc, ic * 128:(ic + 1) * 128], ptr[:])

            # ---- ctx feature-major -> cmpin[:, 4+dc, :] (DR over token chunks) ----
            for s in range(2):
                for dc in range(4):
                    pm = attn_ps(f"ctxT{b}_{s}{dc}")
                    nc.tensor.matmul(pm[:], lhsT=xembs[s][:, :, dc * 128:(dc + 1) * 128],
                                     rhs=attT[:, s * 2:s * 2 + 2, :], start=True, stop=True,
                                     perf_mode=DR)
                    nc.vector.tensor_copy(cmpin[:, 4 + dc, s * 256:(s + 1) * 256], pm[:])

            if STAGE < 6:
                continue
            # ---- inter MLP (input = cmpin kc 0..7, K=1024) ----
            mlp_fm(cmpin, 8, wa1, bsb["b_a1"], hmid, f"im1_{b}")
            mlp_fm(hmid, 4, wa2, bsb["b_a2"], qb, f"im2_{b}")  # qb = [pq | hk]

            if STAGE < 7:
                continue
            # ---- inter attention z = pq @ hk^T ----
            zm = work.tile([128, 2, 256], f32r, tag="zm", bufs=3, name=f"zm{b}")
            p2h = work.tile([128, 2, 256], hf, tag="p2h", bufs=3, name=f"p2h{b}")
            for ic in range(2):
                pz = attn_ps(f"z{b}_{ic}")
                nc.tensor.matmul(pz[:], lhsT=ones[:], rhs=madd[0:1, 1, :], start=True, stop=False,
                                 skip_group_check=True)
                for kp in range(2):
                    nc.tensor.matmul(pz[:], lhsT=qb[:, 2 * kp:2 * kp + 2, ic * 128:(ic + 1) * 128],
                                     rhs=qb[:, 2 * kp:2 * kp + 2, 256:512],
                                     start=False, stop=(kp == 1), perf_mode=DR,
                                     skip_group_check=True)
                nc.vector.tensor_copy(zm[:, ic, :], pz[:])
                softmax_rows(pz[:], p2h[:, ic, :], f"_p2h{b}_{ic}")

            h2p = work.tile([128, 2, 256], hf, tag="h2p", bufs=3, name=f"h2p{b}")
            for jc in range(2):
                pzT = attn_ps(f"zT{b}_{jc}")
                nc.tensor.matmul(pzT[:], lhsT=ones[:], rhs=madd[0:1, 0, :],
                                 start=True, stop=False)
                for ic in range(2):
                    nc.tensor.matmul(pzT[:, ic * 128:(ic + 1) * 128].bitcast(f32r),
                                     lhsT=zm[:, ic, jc * 128:(jc + 1) * 128],
                                     rhs=identr[:], is_transpose=True, start=False, stop=(ic == 1))
                softmax_rows(pzT[:], h2p[:, jc, :], f"_h2p{b}_{jc}")

            if STAGE < 8:
                continue
            p2hT = work.tile([128, 2, 256], f8, tag="p2hT", bufs=3, name=f"p2hT{b}")
            h2pT = work.tile([128, 2, 256], f8, tag="h2pT", bufs=3, name=f"h2pT{b}")
            for srcT, dstT, nm in ((p2h, p2hT, "p"), (h2p, h2pT, "h")):
                for jc in range(2):
                    for ic in range(2):
                        ptr = trT_ps(f"{nm}T{b}_{jc}{ic}", hf)
                        nc.tensor.matmul(ptr[:], lhsT=srcT[:, ic, jc * 128:(jc + 1) * 128],
                                         rhs=identb[:], is_transpose=True, start=True, stop=True)
                        nc.vector.tensor_copy(dstT[:, jc, ic * 128:(ic + 1) * 128], ptr[:])

            if STAGE < 9:
                continue
            # ---- Y = cat @ Wc1_bot (token-major out, feature-major input; DR) ----
            Yt = work.tile([128, 4, 512], f8, tag="Y", bufs=3, name=f"Y{b}")
            for s in range(2):
                for tcn in range(2):
                    pm = mm512_ps(f"Y{b}_{s}{tcn}")
                    for kp in range(4):
                        nc.tensor.matmul(pm[:], lhsT=cmpin[:, 2 * kp:2 * kp + 2, s * 256 + tcn * 128:s * 256 + (tcn + 1) * 128],
                                         rhs=wc1b[:, 2 * kp:2 * kp + 2, :],
                                         start=(kp == 0), stop=(kp == 3), perf_mode=DR)
                    nc.vector.tensor_copy(Yt[:, s * 2 + tcn, :], pm[:])

            # ---- compare L1 (feature-major, both seqs; DR everywhere) ----
            cmp1 = work.tile([128, 4, 512], hf, tag="cmp1", bufs=3, name=f"cmp1_{b}")
            for nf in range(4):
                pm = mm512_ps(f"c1_{b}_nf{nf}")
                for kp in range(4):
                    nc.tensor.matmul(pm[:], lhsT=wc1t[:, kp, nf, :],
                                     rhs=cmpin[:, 2 * kp:2 * kp + 2, :],
                                     start=(kp == 0), stop=False, perf_mode=DRI)
                nc.tensor.matmul(pm[:, 0:256], lhsT=Yt[:, 2:4, nf * 128:(nf + 1) * 128],
                                 rhs=p2hT[:, 0:2, :], start=False, stop=False, perf_mode=DR)
                nc.tensor.matmul(pm[:, 256:512], lhsT=Yt[:, 0:2, nf * 128:(nf + 1) * 128],
                                 rhs=h2pT[:, 0:2, :], start=False, stop=True, perf_mode=DR)
                nc.scalar.activation(cmp1[:, nf, :], pm[:], AF.Relu, bias=bsb["b_c1"][:, nf:nf + 1])

            if STAGE < 10:
                continue
            # ---- compare L2 (token-major, fp16) + masked sum (DR fp8) ----
            for s in range(2):
                cmp2 = work.tile([128, 2, 512], f8, tag="cmp2", bufs=3, name=f"cmp2_{b}_{s}")
                for tcn in range(2):
                    pm = mm512_ps(f"c2_{b}_{s}{tcn}")
                    nc.tensor.matmul(pm[:], lhsT=ones[:], rhs=bc2row[:], start=True, stop=False)
                    for kc in range(4):
                        nc.tensor.matmul(pm[:], lhsT=cmp1[:, kc, s * 256 + tcn * 128:s * 256 + (tcn + 1) * 128],
                                         rhs=wc2[:, kc, :], start=False, stop=(kc == 3))
                    nc.scalar.activation(cmp2[:, tcn, :], pm[:], AF.Relu)
                pa = ps.tile([1, 512], f32, space="PSUM", tag="mm512", bufs=3, name=f"sum{b}_{s}")
                for tcn in range(2):
                    nc.tensor.matmul(pa[:], lhsT=maskf[:, s, tcn, :], rhs=cmp2[:, tcn, :],
                                     start=(tcn == 0), stop=(tcn == 1))
                srow = work.tile([1, 512], f32, tag="sumrow", bufs=3, name=f"srow{b}_{s}")
                nc.vector.tensor_copy(srow[:], pa[:])
                nc.sync.dma_start(srows[s * BL + b:s * BL + b + 1, :].bitcast(f32), srow[:])

            if b == 0 and dbg:
                def tap(name, src_ap):
                    if name in dbg:
                        stg = work.tile(list(dbg[name].shape), f32, tag=f"tap_{name}", bufs=1, name=f"tap{name}")
                        nc.vector.tensor_copy(stg[:], src_ap)
                        nc.sync.dma_start(dbg[name].ap(), stg[:])
                tap("dbg_cmpin", cmpin[:])
                tap("dbg_q", qb[:])
                tap("dbg_att", att[:])
                tap("dbg_zm", zm[:].bitcast(f32))
                tap("dbg_p2h", p2h[:])
                tap("dbg_h2p", h2p[:])
                tap("dbg_Y", Yt[:])
                tap("dbg_cmp1", cmp1[:])

        # ---------------- aggregate MLP (all items at once, fp16) ----------------
        run_agg = (nitems == BL) and STAGE >= 11
        if run_agg:
            aggT = work.tile([128, 2, 4, BL], hf, tag="aggT", bufs=1, name="aggT")
            for dc in range(4):
                ptr = ps.tile([128, 2 * BL], f32r, space="PSUM", tag="trT", bufs=2,
                              name=f"aggTr{dc}")
                nc.tensor.matmul(ptr[:], lhsT=srows[:, dc * 128:(dc + 1) * 128],
                                 rhs=identr[0:2 * BL, 0:2 * BL], is_transpose=True,
                                 start=True, stop=True)
                for s in range(2):
                    nc.vector.tensor_copy(aggT[:, s, dc, :],
                                          ptr[:, s * BL:(s + 1) * BL].bitcast(f32))
            agg1 = work.tile([128, 4, BL], hf, tag="agg1", bufs=1, name="agg1")
            for nf in range(4):
                pm = attn_ps(f"g1_{nf}")
                for kc in range(8):
                    nc.tensor.matmul(pm[:, 0:BL], lhsT=wg1[:, kc, nf * 128:(nf + 1) * 128],
                                     rhs=aggT[:, kc // 4, kc % 4, :], start=(kc == 0), stop=(kc == 7))
                nc.scalar.activation(agg1[:, nf, :], pm[:, 0:BL], AF.Relu, bias=bsb["b_g1"][:, nf:nf + 1])
            agg2 = work.tile([128, 4, BL], hf, tag="agg2", bufs=1, name="agg2")
            for nf in range(4):
                pm = attn_ps(f"g2_{nf}")
                for kc in range(4):
                    nc.tensor.matmul(pm[:, 0:BL], lhsT=wg2[:, kc, nf * 128:(nf + 1) * 128],
                                     rhs=agg1[:, kc, :], start=(kc == 0), stop=(kc == 3))
                nc.scalar.activation(agg2[:, nf, :], pm[:, 0:BL], AF.Relu, bias=bsb["b_g2"][:, nf:nf + 1])
            po = attn_ps("po")
            for kc in range(4):
                nc.tensor.matmul(po[0:BL, 0:4], lhsT=agg2[:, kc, :], rhs=wo[:, kc, :],
                                 start=(kc == 0), stop=(kc == 3))
            osb = work.tile([BL, OUT], f32, tag="osb", bufs=1, name="osb")
            nc.vector.tensor_copy(osb[:], po[0:BL, 0:OUT])
            nc.sync.dma_start(out_d.ap(), osb[:])

        ps.release()
        work.release()
        const.release()

    nc.compile()
    return nc


def _get_program(debug_taps=()):
    key = tuple(n for n, _ in debug_taps)
    if key not in _PROG_CACHE:
        _PROG_CACHE[key] = _build_program(debug_taps)
    return _PROG_CACHE[key]


def kernel(prem_input, hypo_input, embed_W, dist_W,
           Ws1, bs1, Ws2, bs2, Wa1, ba1, Wa2, ba2,
           Wc1, bc1, Wc2, bc2, Wg1, bg1, Wg2, bg2, Wo,
           _debug_taps=(), _trace=False, _tmpdir=None):
    import concourse.mybir as mybir
    from concourse.bass_utils import run_bass_kernel_spmd

    nc = _get_program(_debug_taps)

    f32 = np.float32
    np_f8 = mybir.dt.np(mybir.dt.float8e4)

    def as_hf(a):
        return np.ascontiguousarray(np.asarray(a, f32).astype(np.float16))

    def as_f8(a):
        return np.ascontiguousarray(np.asarray(a, f32).astype(np_f8))

    def as_dri(a):
        """fp8 weight [K, 512] -> DoubleRowSwInterleave stationary layout
        [128, K//256 pairs, 4 nf-chunks, 256]: per 128x128 k-tile pair the
        column pairs (A,B) are interleaved with columns reversed."""
        W = np.asarray(a, f32).astype(np_f8)
        K = W.shape[0]
        t = W.reshape(K // 128, 128, 4, 128)          # [kc, p, nf, m]
        rev = t[:, :, :, ::-1]
        out = np.empty((128, K // 256, 4, 256), np_f8)
        out[:, :, :, 0::2] = rev[0::2].transpose(1, 0, 2, 3)
        out[:, :, :, 1::2] = rev[1::2].transpose(1, 0, 2, 3)
        return np.ascontiguousarray(out)

    Wc1f = np.asarray(Wc1, f32)
    common = {
        "emb": as_f8(embed_W),
        "dW": np.ascontiguousarray(np.asarray(dist_W, f32).reshape(-1)),
        "w_s1": as_dri(Ws1), "w_s2": as_dri(Ws2),
        "w_a1": as_dri(Wa1), "w_a2": as_dri(Wa2),
        "w_c1t": as_dri(Wc1f[:2 * E]), "w_c1b": as_f8(Wc1f[2 * E:]),
        "w_c2": as_hf(Wc2),
        "w_g1": as_hf(Wg1), "w_g2": as_hf(Wg2),
        "w_o": as_hf(Wo),
        "b_s1": np.ascontiguousarray(bs1, f32), "b_s2": np.ascontiguousarray(bs2, f32),
        "b_a1": np.ascontiguousarray(ba1, f32), "b_a2": np.ascontiguousarray(ba2, f32),
        "b_c1": np.ascontiguousarray(bc1, f32), "b_c2": np.ascontiguousarray(bc2, f32),
        "b_g1": np.ascontiguousarray(bg1, f32), "b_g2": np.ascontiguousarray(bg2, f32),
    }
    # int32 transport verified bit-exact in this environment (probed with a
    # round-trip kernel), so indices ship unshifted
    prem = np.ascontiguousarray(np.asarray(prem_input).reshape(B, L).astype(np.int32))
    hypo = np.ascontiguousarray(np.asarray(hypo_input).reshape(B, L).astype(np.int32))

    in_maps = []
    for c in range(NCORES):
        sl = slice(c * BL, (c + 1) * BL)
        tokc = np.stack([prem[sl], hypo[sl]], axis=0)  # [2, BL, L]
        in_maps.append({"tok": np.ascontiguousarray(tokc), **common})

    kwargs = {}
    if _trace:
        kwargs.update(trace=True, tmpdir=_tmpdir)
    res = run_bass_kernel_spmd(nc, in_maps, core_ids=list(range(NCORES)), **kwargs)
    out = np.concatenate([r["out"] for r in res.results], axis=0)
    if _debug_taps or _trace:
        return out, res
    return out
